# revision 1
# baseline (speedup 1.0000x reference)
"""CapsuleTransformConv on 8 Trainium2 NeuronCores.

Problem:  x [4,16,16,32,16] f32, matrix [288,16,512] f32.
          im2col (K=3, VALID) -> tile [4,14,14,288,16]
          votes  = einsum('bhwna,nac->bhwnc', tile, matrix)
          out    = votes.reshape(4,14,14,288,32,16)

Sharding: tensor-parallel over the filter*atom output axis (512 -> 64 per
core).  Every core reads the full x (2 MB) and its 64-wide slice of the
weights; writes its [784, 288, 64] slice of the output (~58 MB, the
dominant HBM traffic).

Per-core kernel (~253 us HW, vs ~208 us pure write time at the measured
~290 GB/s per-core effective HBM write rate with all 8 cores active):
  - x is loaded once (2 DMAs) and PE-transposed into 4 per-octet tiles
    xT[(c_in_octet, atom)=128 partitions, (b,h,w)=1024]; x is read from
    HBM exactly once.
  - Per tap (ki,kj), GPSIMD compacts the im2col gather into
    tap[(dc,a), oct*784 + (b,i,j)] so every matmul's stationary operand
    is a flat contiguous slice (walrus requires a single free dim).
  - Weights for 8 consecutive capsules (one c-octet of one tap) are laid
    out block-diagonally in a [128, 512] f32r tile so one K=128 matmul
    computes 8 independent [pos,16]@[16,64] capsule matmuls.  FP32r
    matmul inputs must be produced by a rounding instruction (never by
    DMA), so paint DMAs land in a reused memset-once f32 buffer and a
    full-partition DVE copy rounds each 4-group chunk into its per-tap
    wpack tile.
  - Main loop: 9 taps x (4 batches x 2 i-windows); each iteration runs
    4 matmuls (c-octets) into one 4-bank PSUM tile, a PSUM->SBUF copy
    split by bank pairs across Vector||Scalar, and one contiguous
    0.7-0.9 MB DMA to the tap-major output, alternating the two HWDGE
    rings.
  - Matmuls run in float32r (TF32-class, 1 cyc/row vs 4 for fp32);
    fp32 accumulation in PSUM; rel err vs fp32 reference ~1.7e-4.
    Set MM_MODE="f32" for bit-exact output at ~303 us.
"""

import numpy as np

B, H, W, C, A = 4, 16, 16, 32, 16
KS = 3
OH = OW = 14
NCAP = KS * KS * C          # 288 capsules
FTOT = 512                  # filter*atom
NCORES = 8
FPC = FTOT // NCORES        # 64 output features per core
POS = B * OH * OW           # 784 output positions
NG = NCAP // 8              # 36 groups of 8 capsules = (tap, c-octet)

_NC_CACHE = {}
MM_MODE = "f32r"  # "f32" (exact, 4 cyc/row) or "f32r" (TF32-class, 1 cyc/row)


def _build_nc(mm_f32r=True):
    import concourse.bass as bass  # noqa: F401
    import concourse.mybir as mybir
    import concourse.tile as tile
    from concourse import bacc, masks

    f32 = mybir.dt.float32
    mmdt = mybir.dt.float32r if mm_f32r else mybir.dt.float32

    nc = bacc.Bacc(None, target_bir_lowering=False)
    x_d = nc.declare_dram_parameter("x", [B, H, W, C, A], f32, isOutput=False)
    m_d = nc.declare_dram_parameter("mat", [NCAP, A, FPC], f32, isOutput=False)
    # Tap-major output layout: out[kk, pos, 32*64].  Each inner-loop DMA then
    # writes one fully contiguous ~0.7-0.9 MB block (vs 8 KB runs strided by
    # 72 KB in pos-major layout); the host transposes kk back into n.
    o_d = nc.declare_dram_parameter("out", [KS * KS, POS, 32 * FPC], f32,
                                    isOutput=True)

    x2d = x_d.rearrange("b h w c a -> (b h w) (c a)")   # [1024, 512]

    with tile.TileContext(nc) as tc:
        with (
            tc.tile_pool(name="const", bufs=1) as constp,
            tc.tile_pool(name="big", bufs=1) as bigp,
            tc.tile_pool(name="stage", bufs=3) as stagep,
            tc.tile_pool(name="tapp", bufs=2) as tapp,
            tc.tile_pool(name="psum", bufs=2, space="PSUM") as psump,
        ):
            ident = constp.tile([128, 128], f32, tag="ident")
            masks.make_identity(nc, ident[:])

            # ---- weights chunk 0 paint: first in the sync ring ----
            # (moved ahead of the x loads; see the wpack build below)
            msrc = m_d.rearrange("(g gc) a f -> gc a g f", gc=8)
            wtmp = bigp.tile([128, 16 * 512], f32, tag="wtmp")
            # Small memset on DVE (idle, early) so round-0 paints go first.
            nc.vector.memset(wtmp[:, 0:2048], 0.0)
            nc.gpsimd.memset(wtmp[:, 2048:], 0.0)
            wtv = wtmp[:].rearrange("p (g v) -> p g v", g=16)
            for gc in range(8):
                nc.sync.dma_start(
                    wtv[gc * 16:(gc + 1) * 16, 0:4, gc * FPC:(gc + 1) * FPC],
                    msrc[gc, :, 0:4, :],
                )

            # ---- x: HBM -> SBUF once, four 2-slab tiles [128, 2, 512] ----
            # (per-batch granularity: batch b's transposes depend only on
            # tile b, so the first matmul chain starts ~2us after the first
            # 512 KB lands)
            xsrc = x2d.rearrange("(t s p) c -> t p s c", t=4, p=128)
            x_sbs = [
                bigp.tile([128, 2 * 512], f32, tag=f"x_sb{t}", name=f"x_sb{t}")
                for t in range(4)
            ]
            for t in range(4):
                nc.sync.dma_start(
                    x_sbs[t][:].rearrange("p (s c) -> p s c", s=2), xsrc[t]
                )

            # ---- weights: block-diagonal wpack, built per-tap ----
            # wpack_c[(gc,a), oct*512 + gc*64 + f] = matrix[(c*4+oct)*8+gc, a, f]
            # else 0.  FP32r matmul inputs must be produced by a rounding
            # instruction (never by DMA), so paint DMAs land in transient f32
            # tiles and a full-partition engine copy rounds each chunk.
            # One chunk per tap kk so kk=0 matmuls start without waiting for
            # the whole weight build.  The two transient tiles are memset
            # once: every chunk paints the same diagonal positions, so the
            # off-diagonal zeros stay clean across reuse.
            # One serially-reused paint buffer covering 4 taps (16 groups);
            # every round paints the same diagonal positions, so the memset
            # zeros stay clean across reuse.  Round 0 (tap 0) was painted
            # above, ahead of the x loads.
            wpacks = []
            for rnd, ntap in ((0, 1), (1, 4), (2, 4)):
                g0 = (0, 4, 20)[rnd]  # first group of this round
                ng = ntap * 4
                if rnd > 0:
                    for gc in range(8):
                        # Scalar ring: idle until outputs begin.
                        nc.scalar.dma_start(
                            wtv[gc * 16:(gc + 1) * 16, 0:ng,
                                gc * FPC:(gc + 1) * FPC],
                            msrc[gc, :, g0: g0 + ng, :],
                        )
                for t in range(ntap):
                    kk_of = g0 // 4 + t
                    wp = bigp.tile(
                        [128, 4 * 512], mmdt,
                        tag=f"wpack{kk_of}", name=f"wpack{kk_of}",
                    )
                    nc.vector.tensor_copy(
                        wp[:], wtmp[:, t * 2048:(t + 1) * 2048]
                    )
                    wpacks.append(wp)

            # ---- xT: PE-transpose x into 4 per-octet tiles [(dc,a), (b,h,w)]
            # Separate tiles so each octet's im2col cast can start as soon as
            # its own 8 transposes land.
            xts = [
                bigp.tile([128, 1024], f32, tag=f"xt{o}", name=f"xt{o}")
                for o in range(4)
            ]
            for s in range(8):
                for oct in range(4):
                    tr = psump.tile([128, 128], f32, tag="mm")
                    nc.tensor.transpose(
                        tr[:],
                        x_sbs[s // 2][
                            :, (s % 2) * 512 + oct * 128:
                            (s % 2) * 512 + (oct + 1) * 128
                        ],
                        ident[:],
                    )
                    dst = xts[oct][:, s * 128:(s + 1) * 128]
                    if (s + oct) % 2 == 0:
                        nc.vector.tensor_copy(dst, tr[:])
                    else:
                        nc.scalar.copy(dst, tr[:])

            xtvs = [
                t[:].rearrange("p (b h w) -> p b h w", b=B, h=H) for t in xts
            ]

            # ---- main loop: 9 taps (outer) x per-batch pos windows ----
            # The matmul stationary operand must be a single flat free dim
            # (walrus constraint), so per tap we compact the im2col gather
            # into tap[(dc,a), oct*784 + (b,i,j)] with GPSIMD copies.
            it = 0
            for kk in range(9):
                ki, kj = kk // 3, kk % 3
                tap = tapp.tile([128, 4 * POS], mmdt, tag="tap")
                for oct in range(4):
                    dst = tap[:, oct * POS:(oct + 1) * POS].rearrange(
                        "p (b i j) -> p b i j", b=B, i=OH
                    )
                    src = xtvs[oct][:, :, ki: ki + OH, kj: kj + OW]
                    if kk == 0:
                        # First tap per-batch on DVE/ACT (idle at startup):
                        # batch b's cast only needs x slabs 2b..2b+1, so the
                        # first matmul starts as soon as the first slabs
                        # transpose.  Later taps prefetch on idle GPSIMD.
                        for bb in range(B):
                            if (oct + bb) % 2 == 0:
                                nc.vector.tensor_copy(
                                    dst[:, bb], src[:, bb]
                                )
                            else:
                                nc.scalar.copy(dst[:, bb], src[:, bb])
                    else:
                        nc.gpsimd.tensor_copy(dst, src)
                for b in range(B):
                    for i0, ni in ((0, 8), (8, 6)):
                        m = ni * OW  # 112 or 84 output positions
                        ps = psump.tile([128, 2048], f32, tag="mm")
                        for oct in range(4):
                            off = oct * POS + b * (OH * OW) + i0 * OW
                            nc.tensor.matmul(
                                ps[0:m, oct * 512:(oct + 1) * 512],
                                tap[:, off: off + m],
                                wpacks[kk][:, oct * 512:(oct + 1) * 512],
                                start=True,
                                stop=True,
                            )
                        st = stagep.tile([128, 2048], f32, tag="st")
                        # Split the PSUM->SBUF copy by bank pairs so DVE and
                        # ACT run in parallel (different PSUM banks).
                        nc.vector.tensor_copy(st[0:m, 0:1024], ps[0:m, 0:1024])
                        nc.scalar.copy(st[0:m, 1024:2048], ps[0:m, 1024:2048])
                        # Alternate the two HWDGE rings (SP / ACT) so output
                        # DMAs pipeline across both.
                        dma_eng = nc.sync if it % 2 == 0 else nc.scalar
                        q0 = b * (OH * OW) + i0 * OW
                        dma_eng.dma_start(
                            o_d[kk, q0: q0 + m, :],
                            st[0:m, :],
                        )
                        it += 1

    nc.compile()
    return nc


def _get_nc():
    key = MM_MODE
    if key not in _NC_CACHE:
        _NC_CACHE[key] = _build_nc(mm_f32r=(MM_MODE == "f32r"))
    return _NC_CACHE[key]


def kernel(x, matrix):
    from concourse.bass_utils import run_bass_kernel_spmd

    x = np.ascontiguousarray(x, dtype=np.float32)
    matrix = np.ascontiguousarray(matrix, dtype=np.float32)
    nc = _get_nc()
    in_maps = [
        {
            "x": x,
            "mat": np.ascontiguousarray(matrix[:, :, c * FPC:(c + 1) * FPC]),
        }
        for c in range(NCORES)
    ]
    r = run_bass_kernel_spmd(nc, in_maps, list(range(NCORES)))
    # parts[c]: [9, 784, 2048] tap-major -> [784, kk, 32, core, 64] -> full
    arr = np.stack([r.results[c]["out"] for c in range(NCORES)])
    arr = arr.reshape(NCORES, KS * KS, POS, 32, FPC)
    arr = arr.transpose(2, 1, 3, 0, 4)               # [pos, kk, 32, core, f]
    full = arr.reshape(POS, NCAP, FTOT)
    return np.ascontiguousarray(
        full.reshape(B, OH, OW, NCAP, 32, 16).astype(np.float32)
    )



# revision 9
# speedup vs baseline: 1.5557x; 1.5557x over previous
"""CapsuleTransformConv on 8 Trainium2 NeuronCores (fp16 pipeline).

Problem:  x [4,16,16,32,16] f32, matrix [288,16,512] f32.
          im2col (K=3, VALID) -> tile [4,14,14,288,16]
          votes  = einsum('bhwna,nac->bhwnc', tile, matrix)
          out    = votes.reshape(4,14,14,288,32,16)

Sharding: tensor-parallel over the filter*atom output axis (512 -> 64 per
core).  Every core reads the full x (2 MB) and its 64-wide weight slice;
writes its 784 x 288 x 64 output slice.

v2 (fp16) design, from the v1 (f32r) trace analysis:
  - Output is written as fp16 (harness gate is rel_err < 2e-2; fp16
    rounding contributes ~5e-4).  Halves the dominant HBM write traffic
    to ~28.9 MB/core.  Host converts back to f32 (free).
  - Weights are block-diagonal-packed ON HOST into wpack[9, 128, 2048]
    fp16 (wpack[kk][(gc,a), oct*512+gc*64+f] = matrix[kk*32+oct*8+gc, a,
    f]); uploaded as a plain contiguous input.  This deletes v1's whole
    memset/paint/cast weight build (which serialized the prologue to
    ~55 us before the first output DMA).  The 9 x 512 KB loads go on the
    GPSIMD SWDGE ring so both HWDGE rings stay free for x + outputs.
  - x is cast f32->fp16 before the PE transposes (fp32 transposes stream
    at 1/4 rate), and matmuls run fp16 x fp16 -> f32 PSUM.
  - Weights-stationary matmuls: stationary = wpack chunk [K=128, M=128
    f-cols], moving = tap [K=128, N=784 positions].  vs v1's
    tap-stationary form this cuts streamed PE columns 147K -> 113K and
    makes every output M=128 wide.  Output becomes f-major
    o[kk, f=2048, pos=784]; the host untangles (free).
  - Per (kk, oct, chunk-pair): 2 matmuls into [128,784] PSUM tiles,
    PSUM->SBUF fp16 copies split DVE/ACT, one 401 KB contiguous DMA
    alternating the two HWDGE rings.
  - Tap compaction (im2col gather) per tap kk>=1: octs 0-1 on GPSIMD,
    oct 2 on DVE, oct 3 on ACT; tap 0 per-batch on DVE/ACT right after
    each batch's transposes so the first matmul fires ~10 us in.
"""

import numpy as np

B, H, W, C, A = 4, 16, 16, 32, 16
KS = 3
OH = OW = 14
NCAP = KS * KS * C          # 288 capsules
FTOT = 512                  # filter*atom
NCORES = 8
FPC = FTOT // NCORES        # 64 output features per core
POS = B * OH * OW           # 784 output positions

_NC_CACHE = {}


def _build_nc():
    import concourse.bass as bass  # noqa: F401
    import concourse.mybir as mybir
    import concourse.tile as tile
    from concourse import bacc, masks

    f32 = mybir.dt.float32
    f16 = mybir.dt.float16
    bf16 = mybir.dt.bfloat16

    nc = bacc.Bacc(None, target_bir_lowering=False)
    x_d = nc.declare_dram_parameter("x", [B, H, W, C, A], f32, isOutput=False)
    w_d = nc.declare_dram_parameter("wpack", [KS * KS, 128, 4 * 512], bf16,
                                    isOutput=False)
    # f-major output: o[kk, f(oct*512+gc*64+f64), pos].  Each inner DMA
    # writes one fully contiguous 401 KB block; host untangles kk/f.
    o_d = nc.declare_dram_parameter("out", [KS * KS, 2048, POS], f16,
                                    isOutput=True)

    x2d = x_d.rearrange("b h w c a -> (b h w) (c a)")   # [1024, 512]
    ov = o_d.rearrange("k (g p) q -> k g p q", p=128)   # [9, 16, 128, 784]

    with tile.TileContext(nc) as tc:
        with (
            tc.tile_pool(name="const", bufs=1) as constp,
            tc.tile_pool(name="big", bufs=1) as bigp,
            tc.tile_pool(name="stage", bufs=4) as stagep,
            tc.tile_pool(name="tapp", bufs=2) as tapp,
            tc.tile_pool(name="psumtr", bufs=2, space="PSUM") as psumtr,
            tc.tile_pool(name="psummm", bufs=4, space="PSUM") as psummm,
        ):
            ident = constp.tile([128, 128], bf16, tag="ident")
            masks.make_identity(nc, ident[:])

            # ---- weights: 9 contiguous 512 KB fp16 loads on SWDGE ----
            wps = [
                bigp.tile([128, 4 * 512], bf16, tag=f"wp{kk}", name=f"wp{kk}")
                for kk in range(9)
            ]
            for kk in range(9):
                nc.gpsimd.dma_start(wps[kk][:], w_d[kk])

            # ---- x: HBM -> SBUF, four [128, 2*512] f32 tiles ----
            # tile t = batch t (rows t*256..t*256+255 of x2d).
            xsrc = x2d.rearrange("(t s p) c -> t p s c", t=4, p=128)
            x_sbs = [
                bigp.tile([128, 2 * 512], f32, tag=f"x_sb{t}", name=f"x_sb{t}")
                for t in range(4)
            ]
            for t in range(4):
                eng = nc.sync if t % 2 == 0 else nc.scalar
                eng.dma_start(
                    x_sbs[t][:].rearrange("p (s c) -> p s c", s=2), xsrc[t]
                )

            # ---- cast x to fp16, then PE-transpose into per-octet tiles
            # xt[oct][(dc,a), (b,h,w)] ----
            x16s = [
                bigp.tile([128, 2 * 512], bf16, tag=f"x16_{t}", name=f"x16_{t}")
                for t in range(4)
            ]
            xts = [
                bigp.tile([128, 1024], bf16, tag=f"xt{o}", name=f"xt{o}")
                for o in range(4)
            ]
            xtvs = [
                t[:].rearrange("p (b h w) -> p b h w", b=B, h=H) for t in xts
            ]
            tap0 = tapp.tile([128, 4 * POS], bf16, tag="tap")
            t0v = [
                tap0[:, o * POS:(o + 1) * POS].rearrange(
                    "p (b i j) -> p b i j", b=B, i=OH
                )
                for o in range(4)
            ]
            for t in range(4):
                if t % 2 == 0:
                    nc.vector.tensor_copy(x16s[t][:], x_sbs[t][:])
                else:
                    nc.scalar.copy(x16s[t][:], x_sbs[t][:])
                for s in (2 * t, 2 * t + 1):
                    for oct in range(4):
                        tr = psumtr.tile([128, 128], bf16, tag="tr")
                        nc.tensor.transpose(
                            tr[:],
                            x16s[t][
                                :, (s % 2) * 512 + oct * 128:
                                (s % 2) * 512 + (oct + 1) * 128
                            ],
                            ident[:],
                        )
                        dst = xts[oct][:, s * 128:(s + 1) * 128]
                        if (s + oct) % 2 == 0:
                            nc.vector.tensor_copy(dst, tr[:])
                        else:
                            nc.scalar.copy(dst, tr[:])
                # batch t of tap 0 compacts as soon as its transposes land
                for oct in range(4):
                    src = xtvs[oct][:, t:t + 1, 0:OH, 0:OW]
                    if (t + oct) % 2 == 0:
                        nc.vector.tensor_copy(t0v[oct][:, t:t + 1], src)
                    else:
                        nc.scalar.copy(t0v[oct][:, t:t + 1], src)

            # ---- main loop: 9 taps x 4 octs x 2 chunk-pairs ----
            it = 0
            for kk in range(9):
                ki, kj = kk // 3, kk % 3
                if kk == 0:
                    tap = tap0
                else:
                    tap = tapp.tile([128, 4 * POS], bf16, tag="tap")
                    for oct in range(4):
                        dst = tap[:, oct * POS:(oct + 1) * POS].rearrange(
                            "p (b i j) -> p b i j", b=B, i=OH
                        )
                        src = xtvs[oct][:, :, ki: ki + OH, kj: kj + OW]
                        if oct < 2:
                            nc.gpsimd.tensor_copy(dst, src)
                        elif oct == 2:
                            nc.vector.tensor_copy(dst, src)
                        else:
                            nc.scalar.copy(dst, src)
                for oct in range(4):
                    for ch in range(4):
                        st = stagep.tile([128, POS], f16, tag="st")
                        wchunk = wps[kk][
                            :, oct * 512 + ch * 128:
                            oct * 512 + (ch + 1) * 128
                        ]
                        # PSUM bank limit: 512 f32/partition, so each
                        # chunk runs as two N=392 matmuls; the fp16
                        # PSUM->SBUF casts then split DVE | ACT.
                        for q in range(2):
                            ps = psummm.tile([128, 392], f32, tag="mm")
                            nc.tensor.matmul(
                                ps[:],
                                wchunk,
                                tap[:, oct * POS + q * 392:
                                    oct * POS + (q + 1) * 392],
                                start=True,
                                stop=True,
                            )
                            dst = st[:, q * 392:(q + 1) * 392]
                            if q == 0:
                                nc.vector.tensor_copy(dst, ps[:])
                            else:
                                nc.scalar.copy(dst, ps[:])
                        dma_eng = nc.sync if it % 2 == 0 else nc.scalar
                        dma_eng.dma_start(ov[kk, oct * 4 + ch], st[:])
                        it += 1

    nc.compile()
    return nc


def _get_nc():
    if "nc" not in _NC_CACHE:
        _NC_CACHE["nc"] = _build_nc()
    return _NC_CACHE["nc"]


def _pack_weights(matrix):
    """matrix [288,16,512] f32 -> per-core block-diag wpack [8][9,128,2048]
    fp16.  wpack[c][kk, gc*16+a, oct*512+gc*64+f] =
    matrix[kk*32+oct*8+gc, a, c*64+f]."""
    m = matrix.reshape(KS * KS, 4, 8, A, NCORES, FPC)  # [kk,oct,gc,a,core,f]
    import ml_dtypes
    out = np.zeros((NCORES, KS * KS, 128, 2048), dtype=ml_dtypes.bfloat16)
    for gc in range(8):
        # rows gc*16..gc*16+16, cols oct*512+gc*64..+64
        blk = m[:, :, gc].astype(ml_dtypes.bfloat16)    # [kk,oct,a,core,f]
        for oct in range(4):
            out[:, :, gc * A:(gc + 1) * A,
                oct * 512 + gc * FPC: oct * 512 + (gc + 1) * FPC] = (
                blk[:, oct].transpose(2, 0, 1, 3)      # [core,kk,a,f]
            )
    return out


def _core_inputs(x, matrix):
    x = np.ascontiguousarray(x, dtype=np.float32)
    wp = _pack_weights(np.asarray(matrix, dtype=np.float32))
    return [
        {"x": x, "wpack": np.ascontiguousarray(wp[c])}
        for c in range(NCORES)
    ]


def _unscramble(parts):
    """parts: [8][9, 2048, 784] fp16 -> [4,14,14,288,32,16] f32."""
    arr = np.stack(parts)                              # [core,kk,col,pos]
    arr = arr.reshape(NCORES, KS * KS, 4, 8, FPC, POS)
    arr = arr.transpose(5, 1, 2, 3, 0, 4)              # [pos,kk,oct,gc,core,f]
    full = arr.reshape(POS, NCAP, FTOT).astype(np.float32)
    return np.ascontiguousarray(
        full.reshape(B, OH, OW, NCAP, 32, 16)
    )


def kernel(x, matrix):
    from concourse.bass_utils import run_bass_kernel_spmd

    nc = _get_nc()
    in_maps = _core_inputs(x, matrix)
    r = run_bass_kernel_spmd(nc, in_maps, list(range(NCORES)))
    return _unscramble([r.results[c]["out"] for c in range(NCORES)])
